# revision 4
# baseline (speedup 1.0000x reference)
"""Trainium2 bass kernel for the GNN message-passing problem.

kernel(**inputs) -> np.ndarray [100000, 1]

Strategy: edges are sharded contiguously across the 8 NeuronCores
(200k/core). The heavy per-edge compute - the 4->128->128->1 teacher MLP
evaluated in both edge directions (silu nonlinearities, ~68 KFLOP/edge,
109 GFLOP total) - runs on the NeuronCores: per 12-column group of 128
edge-partitions, the [r|v_ij] features are transposed via the PE
(3 columns per 128-wide window at partition bases {0,32,64}), then
matmul W0 -> silu -> matmul W1 -> silu -> per-column matmul with w2
(h1 as the stationary operand) produce per-edge messages m_i, m_j.
r = |r_ij|/H is computed on-device in a prologue (single sqrt table
load). The index-dependent segment-sum/count/divide runs on the host
(np.add.at / bincount), as does the v[i]-v[j] gather prep.
"""
import sys
sys.path.insert(0, "/opt/trn_rl_repo")
import numpy as np

N_NODES = 100000
E_TOTAL = 1600000
HSM = 3.0
N_CORES = 8
CC = 96          # columns per chunk
GCOLS = 12       # columns per MLP group
NW = GCOLS // 3  # transpose windows per group


def _split_multi_waits(nc, max_waits=1):
    """This walrus build rejects >1 sync-wait on CTRL ops (Tile's final
    drain). Move extra waits onto preceding single-wait InstNoOps."""
    import concourse.mybir as mybir
    n_split = 0
    for f in nc.m.functions:
        for blk in f.blocks:
            insns = blk.instructions
            out = []
            for ins in insns:
                si = ins.sync_info
                if si is not None and si.on_wait and len(si.on_wait) > max_waits:
                    waits = list(si.on_wait)
                    for k, w in enumerate(waits[:-max_waits]):
                        nop = mybir.InstNoOp(name=f"{ins.name}-ws{k}")
                        nop.engine = ins.engine
                        nop.sync_info = mybir.SyncInfo(on_wait=[w], on_update=[])
                        out.append(nop)
                        n_split += 1
                    si.on_wait = waits[-max_waits:]
                out.append(ins)
            blk.instructions = out
    return n_split


def _build_kernel(K, b2val):
    import concourse.bass as bass
    import concourse.mybir as mybir
    from concourse.tile import TileContext
    from concourse.masks import make_identity

    F32 = mybir.dt.float32
    AF = mybir.ActivationFunctionType
    OP = mybir.AluOpType
    T = K // CC
    NG = CC // GCOLS
    GE = GCOLS * 128

    nc = bass.Bass()
    x4_d = nc.declare_dram_parameter("x4", [128, K, 4], F32, isOutput=False)
    r4_d = nc.declare_dram_parameter("r4", [128, K, 4], F32, isOutput=False)
    w0_d = nc.declare_dram_parameter("w0rep", [128, 128], F32, isOutput=False)
    w1_d = nc.declare_dram_parameter("w1lt", [128, 128], F32, isOutput=False)
    w2_d = nc.declare_dram_parameter("w2c", [128, 1], F32, isOutput=False)
    b0_d = nc.declare_dram_parameter("b0c", [128, 1], F32, isOutput=False)
    b1_d = nc.declare_dram_parameter("b1c", [128, 1], F32, isOutput=False)
    mi_d = nc.declare_dram_parameter("mi", [128, K], F32, isOutput=True)
    mj_d = nc.declare_dram_parameter("mj", [128, K], F32, isOutput=True)

    with TileContext(nc) as tc:
        with (
            tc.tile_pool(name="const", bufs=1) as cpool,
            tc.tile_pool(name="io", bufs=3) as iop,
            tc.tile_pool(name="xbuf", bufs=2) as xp,
            tc.tile_pool(name="tsp", bufs=3) as tsp,
            tc.tile_pool(name="hid", bufs=3) as hp,
            tc.tile_pool(name="mp", bufs=2) as mp,
            tc.tile_pool(name="psAux", bufs=2, space="PSUM") as psA,
            tc.tile_pool(name="psBig", bufs=2, space="PSUM") as psB,
        ):
            ident = cpool.tile([128, 128], F32)
            make_identity(nc, ident[:])
            w0t = cpool.tile([128, 128], F32, tag="w0")
            w1t = cpool.tile([128, 128], F32, tag="w1")
            w2t = cpool.tile([128, 1], F32, tag="w2")
            b0t = cpool.tile([128, 1], F32, tag="b0")
            b1t = cpool.tile([128, 1], F32, tag="b1")
            for tgt, src in ((w0t, w0_d), (w1t, w1_d), (w2t, w2_d),
                             (b0t, b0_d), (b1t, b1_d)):
                nc.sync.dma_start(out=tgt[:], in_=src[:])

            # prologue: r for all chunks (one sqrt table load)
            r_all = cpool.tile([128, K], F32, tag="rall")
            for t in range(T):
                sl = slice(t * CC, (t + 1) * CC)
                r4t = iop.tile([128, CC * 4], F32, tag="r4t")
                nc.sync.dma_start(
                    out=r4t[:],
                    in_=r4_d[:, sl, :].rearrange("p c f -> p (c f)"))
                sq = iop.tile([128, CC * 4], F32, tag="sq")
                nc.vector.tensor_tensor(out=sq[:], in0=r4t[:], in1=r4t[:],
                                        op=OP.mult)
                rs = iop.tile([128, CC], F32, tag="rs")
                nc.vector.tensor_tensor(out=rs[:], in0=sq[:, 1::4],
                                        in1=sq[:, 2::4], op=OP.add)
                nc.vector.tensor_tensor(out=rs[:], in0=rs[:],
                                        in1=sq[:, 3::4], op=OP.add)
                nc.scalar.activation(out=r_all[:, sl], in_=rs[:], func=AF.Sqrt,
                                     scale=float(1.0 / (HSM * HSM)))

            for t in range(T):
                sl = slice(t * CC, (t + 1) * CC)
                X = xp.tile([128, CC * 4], F32, tag="X")
                nc.sync.dma_start(
                    out=X[:], in_=x4_d[:, sl, :].rearrange("p c f -> p (c f)"))
                Xn = xp.tile([128, CC * 4], F32, tag="Xn")
                nc.vector.tensor_scalar_mul(out=Xn[:], in0=X[:], scalar1=-1.0)
                nc.vector.tensor_copy(out=X[:, 0::4], in_=r_all[:, sl])
                nc.vector.tensor_copy(out=Xn[:, 0::4], in_=r_all[:, sl])

                m_i = mp.tile([128, CC], F32, tag="mi")
                m_j = mp.tile([128, CC], F32, tag="mj")

                for g in range(NG):
                    for src, mt, dirn in ((X, m_i, "i"), (Xn, m_j, "j")):
                        tps = psB.tile([128, GE], F32, tag="big")
                        for cci in range(GCOLS):
                            nc.tensor.transpose(
                                out=tps[0:4, cci * 128:(cci + 1) * 128],
                                in_=src[:, (g * GCOLS + cci) * 4:
                                        (g * GCOLS + cci) * 4 + 4],
                                identity=ident[:])
                        ts = tsp.tile([4, GE], F32, tag="ts")
                        nc.vector.tensor_copy(out=ts[:], in_=tps[0:4, :])

                        p0 = psB.tile([128, GE], F32, tag="big")
                        for cci in range(GCOLS):
                            nc.tensor.matmul(
                                out=p0[:, cci * 128:(cci + 1) * 128],
                                lhsT=w0t[0:4, :],
                                rhs=ts[:, cci * 128:(cci + 1) * 128],
                                start=True, stop=True)
                        h0 = hp.tile([128, GE], F32, tag="h0")
                        nc.scalar.activation(out=h0[:], in_=p0[:],
                                             func=AF.Silu, bias=b0t[:])
                        p1 = psB.tile([128, GE], F32, tag="big")
                        for q in range(GE // 512):
                            nc.tensor.matmul(
                                out=p1[:, q * 512:(q + 1) * 512],
                                lhsT=w1t[:],
                                rhs=h0[:, q * 512:(q + 1) * 512],
                                start=True, stop=True)
                        h1 = hp.tile([128, GE], F32, tag="h1")
                        nc.scalar.activation(out=h1[:], in_=p1[:],
                                             func=AF.Silu, bias=b1t[:])
                        pm = psA.tile([128, GCOLS], F32, tag="aux")
                        for cci in range(GCOLS):
                            nc.tensor.matmul(
                                out=pm[:, cci:cci + 1],
                                lhsT=h1[:, cci * 128:(cci + 1) * 128],
                                rhs=w2t[:], start=True, stop=True)
                        nc.vector.tensor_scalar_add(
                            out=mt[:, GCOLS * g:GCOLS * (g + 1)],
                            in0=pm[:], scalar1=b2val)

                nc.sync.dma_start(out=mi_d[:, sl], in_=m_i[:])
                nc.sync.dma_start(out=mj_d[:, sl], in_=m_j[:])
    return nc


def prepare(v, r_ij, W0, b0, W1, b1, W2, b2, edge_index):
    """Host prep: returns (nc, in_maps, postprocess_fn)."""
    v = np.asarray(v, np.float32)
    r_ij = np.asarray(r_ij, np.float32)
    ei = np.asarray(edge_index)
    i_all = ei[0].astype(np.int64)
    j_all = ei[1].astype(np.int64)
    E = i_all.shape[0]
    Epc = E // N_CORES

    # columns per core, padded to chunk multiples
    K = ((Epc + 127) // 128 + CC - 1) // CC * CC

    w0rep = np.zeros((128, 128), np.float32)
    W0 = np.asarray(W0, np.float32)
    for u in range(3):
        w0rep[32 * u:32 * u + 4, :] = W0.T
    wmap = {
        "w0rep": w0rep,
        "w1lt": np.ascontiguousarray(np.asarray(W1, np.float32).T),
        "w2c": np.ascontiguousarray(np.asarray(W2, np.float32).T),
        "b0c": np.asarray(b0, np.float32).reshape(128, 1),
        "b1c": np.asarray(b1, np.float32).reshape(128, 1),
    }
    b2val = float(np.asarray(b2).reshape(()))

    vij_all = v[i_all] - v[j_all]          # [E, 3] gather-diff
    in_maps = []
    for c in range(N_CORES):
        sl = slice(c * Epc, (c + 1) * Epc)
        x4 = np.zeros((128 * K, 4), np.float32)
        x4[:Epc, 1:4] = vij_all[sl]
        r4 = np.zeros((128 * K, 4), np.float32)
        r4[:Epc, 1:4] = r_ij[sl]
        m = {
            "x4": x4.reshape(128, K, 4),   # slot s=(p*K+cpos): edge c*Epc+s
            "r4": r4.reshape(128, K, 4),
        }
        m.update(wmap)
        in_maps.append(m)

    nc = _build_kernel(K, b2val)
    _split_multi_waits(nc)

    def post(results):
        S_i = np.zeros((N_NODES, 1), np.float64)
        S_j = np.zeros((N_NODES, 1), np.float64)
        c_i = np.maximum(np.bincount(i_all, minlength=N_NODES), 1.0)
        c_j = np.maximum(np.bincount(j_all, minlength=N_NODES), 1.0)
        for c in range(N_CORES):
            mi = np.asarray(results[c]["mi"]).reshape(-1)[:Epc]
            mj = np.asarray(results[c]["mj"]).reshape(-1)[:Epc]
            sl = slice(c * Epc, (c + 1) * Epc)
            np.add.at(S_i[:, 0], i_all[sl], mi)
            np.add.at(S_j[:, 0], j_all[sl], mj)
        S = S_i / c_i[:, None] + S_j / c_j[:, None]
        return S.astype(np.float32)

    return nc, in_maps, post


def kernel(v, r_ij, W0, b0, W1, b1, W2, b2, edge_index):
    from concourse.bass_utils import run_bass_kernel_spmd
    nc, in_maps, post = prepare(v, r_ij, W0, b0, W1, b1, W2, b2,
                                edge_index)
    res = run_bass_kernel_spmd(nc, in_maps, core_ids=list(range(N_CORES)))
    return post(res.results)
